# revision 26
# baseline (speedup 1.0000x reference)
"""Multi-head self-attention (AdaptiveTemporalContrastEnhancement) on 8 TRN2 cores.

Key facts baked in:
- The temporal-difference bias delta_c is added uniformly along the softmax
  axis, so softmax cancels it exactly -> it is skipped entirely.
- max |logit| ~ 1.9, so softmax runs without max-subtraction.
- V bias + output bias fold into one effective output bias:
      out = A@(XWv^T + bv) Wo^T + bo = A@(XWv^T)Wo^T + (Wo bv + bo).
- 1/sqrt(dh) is folded into WQT/BQ host-side.
- Data parallel over the 16 (b,t) slices: 2 slices per core, no collectives.
- All matmuls in bf16 (1 cyc/row on PE); accumulation fp32 in PSUM.

Device layout per slice (all "T" = dim-major, tokens along the free axis):
  XT  [d, n]   : 4 x [128, 1024] sbuf tiles (host pre-transposed)
  QT,KT [e, n] : computed as W^T.T @ XT  (4 x [128,1024])
  V_pad [n, .] : token-major, padded per head to a [128,128] stationary:
                 even head h: V cols 0-63, ones col 64, zeros 65-127
                 odd  head h: ones col 0, zeros 1-63,  V cols 64-127
                 so the PV matmul puts head h's Z^T at partitions 64*(h%2)..+63
                 and the softmax denominator at row 64 (even) / row 0 (odd).
  S^T [kv, q]  : head-PAIR packed: one [128, 1024] psum tile holds both heads'
                 S^T for one (kv, q-half); the two S matmuls use disjoint PE
                 row groups (partitions 0-63 / 64-127) and run concurrently.
  Z^T [d, q]   : per (head, q-half) [128, 512] psum accum over kv; evacuated
                 (with denominator row) to sbuf zun per head block.
  denominators : reshaped by DMA into dall8[128, 64] (head h = 16 partitions
                 x 64 cols) so ONE short-free-dim reciprocal per head PAIR is
                 cheap; broadcast back via a DRAM bounce; TT-mult per head.
  O^T [e, n]   : out-proj from normalized Z^T; host transposes back.

The schedule is software-pipelined at instruction level: projection and
out-projection chunks are emitted between attention head-pairs so the PE
fills the idle left by the ACT-paced exp stream, keeping the PE busy (and
its HAM clock-gate warm) while both slices' attention runs back-to-back.
"""

import os
import numpy as np
import ml_dtypes

B, T, N, D = 2, 8, 1024, 512
H, DH = 8, 64
P = 128
NDT = D // P          # 4 d-tiles
NKV = N // P          # 8 kv tiles
NQH = N // 512        # 2 q halves
NCORES = 8
NSLICE = (B * T) // NCORES   # 2 slices per core
S_SCALE = float(1.0 / np.sqrt(DH))  # 0.125

_CACHE = {}


def _build_nc():
    import concourse.mybir as mybir
    from concourse import bacc
    from concourse.tile import TileContext
    import concourse.bass as bass

    f32, bf16 = mybir.dt.float32, mybir.dt.bfloat16
    nc = bacc.Bacc("TRN2", target_bir_lowering=False, debug=False)

    XT = nc.dram_tensor("XT", [NSLICE, D, N], bf16, kind="ExternalInput")
    WQT = nc.dram_tensor("WQT", [D, D], bf16, kind="ExternalInput")
    WKT = nc.dram_tensor("WKT", [D, D], bf16, kind="ExternalInput")
    WVT = nc.dram_tensor("WVT", [D, D], bf16, kind="ExternalInput")
    WOT = nc.dram_tensor("WOT", [D, D], bf16, kind="ExternalInput")
    BQ = nc.dram_tensor("BQ", [NDT, P, 1], f32, kind="ExternalInput")
    BK = nc.dram_tensor("BK", [NDT, P, 1], f32, kind="ExternalInput")
    BO = nc.dram_tensor("BO", [NDT, P, 1], f32, kind="ExternalInput")
    OT = nc.dram_tensor("OT", [NSLICE, D, N], f32, kind="ExternalOutput")

    Exp = mybir.ActivationFunctionType.Exp
    Mult = mybir.AluOpType.mult

    with TileContext(nc) as tc:
        with (
            tc.tile_pool(name="wpool", bufs=1) as wpool,
            tc.tile_pool(name="xpool", bufs=2) as xpool,
            tc.tile_pool(name="qkpool", bufs=2) as qkpool,
            tc.tile_pool(name="vpool", bufs=2) as vpool,
            tc.tile_pool(name="apool", bufs=4) as apool,
            tc.tile_pool(name="zpool", bufs=2) as zpool,
            tc.tile_pool(name="rpool", bufs=2) as rpool,
            tc.tile_pool(name="opool", bufs=3) as opool,
            tc.tile_pool(name="drpool", bufs=2, space="DRAM") as drpool,
            tc.tile_pool(name="ps_s", bufs=2, space="PSUM") as ps_s,
            tc.tile_pool(name="ps_z", bufs=2, space="PSUM") as ps_z,
            tc.tile_pool(name="ps_c", bufs=1, space="PSUM") as ps_c,
        ):
            # ---- persistent weights / biases (DMAs split for queue parallelism;
            #      wq/wk/bq/bk first so the first projection chunk starts early) ----
            w_sb, b_sb = {}, {}

            def emit_w(name, dram):
                # one 3D-AP DMA per weight: [512,512] dram -> [128, 4*512] sbuf
                t = wpool.tile([P, NDT * 512], bf16, tag=name, name=f"w_{name}")
                w_sb[name] = t
                nc.sync.dma_start(
                    out=t[:, :].rearrange("p (dt e) -> p dt e", e=512),
                    in_=dram[:, :].rearrange("(dt p) e -> p dt e", p=P),
                )

            def emit_b(name, dram):
                t = wpool.tile([P, NDT], f32, tag=name, name=f"b_{name}")
                b_sb[name] = t
                nc.sync.dma_start(
                    out=t[:, :],
                    in_=dram[:, :, :].rearrange("et p one -> p (et one)"),
                )

            def load_x(s):
                # one 1MB DMA: large transfers reach full fabric bandwidth
                xt = xpool.tile([P, NDT * N], bf16, tag="xt", name=f"xt_{s}")
                nc.sync.dma_start(
                    out=xt[:, :].rearrange("p (dt n) -> p dt n", n=N),
                    in_=XT[s].rearrange("(dt p) n -> p dt n", p=P),
                )
                return xt

            def gen_qk_chunk(s, et, xt, qt, kt):
                """Filler generator: yields after each matmul so attention
                can weave these into the exp-paced stream one MM at a time."""
                for dst, wname, bname in ((qt[et], "wq", "bq"), (kt[et], "wk", "bk")):
                    w = w_sb[wname]
                    ps = ps_c.tile([P, N], f32, tag="c", name=f"psc_{wname}_{s}_{et}")
                    for dt_ in range(NDT):
                        for qh in range(NQH):
                            nc.tensor.matmul(
                                ps[:, qh * 512:(qh + 1) * 512],
                                lhsT=w[:, dt_ * 512 + et * P: dt_ * 512 + (et + 1) * P],
                                rhs=xt[:, dt_ * N + qh * 512: dt_ * N + qh * 512 + 512],
                                start=(dt_ == 0), stop=(dt_ == NDT - 1),
                            )
                            if dt_ == NDT - 1:
                                # evac inside the same pop as the last matmul so
                                # consumers emitted next step see it ordered
                                nc.vector.tensor_scalar_add(
                                    dst[:, qh * 512: qh * 512 + 512],
                                    ps[:, qh * 512:(qh + 1) * 512],
                                    b_sb[bname][:, et:et + 1],
                                )
                            yield

            def gen_op_chunk(s, et, zt):
                ps = ps_c.tile([P, N], f32, tag="c", name=f"psc_o_{s}_{et}")
                o_sb = opool.tile([P, N], f32, tag="o", name=f"o_{s}_{et}")
                for dd in range(NDT):
                    for qh in range(NQH):
                        nc.tensor.matmul(
                            ps[:, qh * 512:(qh + 1) * 512],
                            lhsT=w_sb["wo"][:, dd * 512 + et * P: dd * 512 + (et + 1) * P],
                            rhs=zt[dd][:, qh * 512: qh * 512 + 512],
                            start=(dd == 0), stop=(dd == NDT - 1),
                        )
                        if dd == NDT - 1:
                            nc.vector.tensor_scalar_add(
                                o_sb[:, qh * 512:(qh + 1) * 512],
                                ps[:, qh * 512:(qh + 1) * 512], b_sb["bo"][:, et:et + 1])
                            if qh == NQH - 1:
                                nc.sync.dma_start(
                                    out=OT[s, et * P:(et + 1) * P, :], in_=o_sb)
                        yield

            def proj_qk_chunk(s, et, xt, qt, kt):
                for dst, wname, bname in ((qt[et], "wq", "bq"), (kt[et], "wk", "bk")):
                    w = w_sb[wname]
                    ps = ps_s.tile([P, N], f32, tag="s", name=f"ps_{wname}_{s}_{et}")
                    for dt_ in range(NDT):
                        for qh in range(NQH):
                            nc.tensor.matmul(
                                ps[:, qh * 512:(qh + 1) * 512],
                                lhsT=w[:, dt_ * 512 + et * P: dt_ * 512 + (et + 1) * P],
                                rhs=xt[:, dt_ * N + qh * 512: dt_ * N + qh * 512 + 512],
                                start=(dt_ == 0), stop=(dt_ == NDT - 1),
                            )
                    for qh in range(NQH):
                        nc.vector.tensor_scalar_add(
                            dst[:, qh * 512: qh * 512 + 512],
                            ps[:, qh * 512:(qh + 1) * 512],
                            b_sb[bname][:, et:et + 1],
                        )

            def proj_v(s, xt):
                v_sb = vpool.tile([P, NKV * H * P], bf16, tag="v", name=f"v_{s}")
                vz = v_sb.rearrange("p (b r) -> p b r", r=256)
                nc.gpsimd.memset(vz[:, :, 65:128], 0.0)    # even-head pad
                nc.gpsimd.memset(vz[:, :, 129:192], 0.0)   # odd-head pad
                nc.vector.memset(vz[:, :, 64:65], 1.0)     # even-head ones col
                nc.vector.memset(vz[:, :, 128:129], 1.0)   # odd-head ones col
                for kv in range(NKV):
                    ps = ps_s.tile([P, N], f32, tag="s", name=f"ps_v_{s}_{kv}")
                    for dt_ in range(NDT):
                        nc.tensor.matmul(
                            ps[:, 0:512],
                            lhsT=xt[:, dt_ * N + kv * P: dt_ * N + (kv + 1) * P],
                            rhs=w_sb["wv"][:, dt_ * 512:(dt_ + 1) * 512],
                            start=(dt_ == 0), stop=(dt_ == NDT - 1),
                        )
                    vblk = v_sb[:, kv * 1024:(kv + 1) * 1024].rearrange(
                        "p (hp r) -> p hp r", r=256)
                    psh = ps[:, 0:512].rearrange("p (hp c) -> p hp c", c=128)
                    nc.vector.tensor_copy(vblk[:, :, 0:64], psh[:, :, 0:64])
                    nc.vector.tensor_copy(vblk[:, :, 192:256], psh[:, :, 64:128])
                return v_sb

            def attention_pair(s, j, qt, kt, v_sb, zun, dall8, filler=None):
                """Heads 2j, 2j+1: S matmuls packed into disjoint PE row
                groups; one exp covers both heads; PV per head/q-half. One
                filler matmul is woven in after each kv step."""
                et = j
                for qh in range(NQH):
                    zs = [ps_z.tile([P, 512], f32, tag="z", name=f"z_{s}_{j}_{qh}_{p_}")
                          for p_ in range(2)]
                    for kv in range(NKV):
                        s_ps = ps_s.tile([P, N], f32, tag="s", name=f"s_{s}_{j}_{qh}_{kv}")
                        for p_ in range(2):
                            pb = 64 * p_
                            nc.tensor.matmul(
                                s_ps[:, p_ * 512:(p_ + 1) * 512],
                                lhsT=kt[et][pb:pb + 64, kv * P:(kv + 1) * P],
                                rhs=qt[et][pb:pb + 64, qh * 512: qh * 512 + 512],
                                start=True, stop=True,
                            )
                        at = apool.tile([P, N], bf16, tag="at", name=f"at_{s}_{j}_{qh}_{kv}")
                        nc.scalar.activation(at, s_ps, Exp)
                        for p_ in range(2):
                            h = 2 * j + p_
                            nc.tensor.matmul(
                                zs[p_],
                                lhsT=v_sb[:, kv * 1024 + h * P: kv * 1024 + (h + 1) * P],
                                rhs=at[:, p_ * 512:(p_ + 1) * 512],
                                start=(kv == 0), stop=(kv == NKV - 1),
                            )
                        if filler is not None:
                            next(filler, None)
                    for p_ in range(2):
                        h = 2 * j + p_
                        nc.vector.tensor_copy(
                            zun[:, h * N + qh * 512: h * N + qh * 512 + 512], zs[p_])
                for p_ in range(2):
                    h = 2 * j + p_
                    dr = 64 if h % 2 == 0 else 0
                    # reshape-gather the denom row into dall8[16h:16h+16, 0:64]
                    nc.gpsimd.dma_start(  # gpsimd: casting DMA bf16 -> f32
                        out=dall8[16 * h:16 * (h + 1), :],
                        in_=zun[dr:dr + 1, h * N:(h + 1) * N])

            def norm_pair(s, j, zun, dall8, rall8, rdram, zt):
                # short-free-dim reciprocal over the pair's 32 partitions
                nc.vector.reciprocal(rall8[32 * j:32 * (j + 1), :],
                                     dall8[32 * j:32 * (j + 1), :])
                rb16 = rpool.tile([P, 64], bf16, tag="rb16", name=f"rb16_{s}_{j}")
                nc.vector.tensor_copy(rb16[32 * j:32 * (j + 1), :],
                                      rall8[32 * j:32 * (j + 1), :])
                nc.sync.dma_start(out=rdram[32 * j:32 * (j + 1), :],
                                  in_=rb16[32 * j:32 * (j + 1), :])
                rbc = rpool.tile([P, N], bf16, tag="rbc", name=f"rbc_{s}_{j}")
                for p_ in range(2):
                    h = 2 * j + p_
                    base = rdram[0:1, 0:1]
                    nc.sync.dma_start(
                        out=rbc[64 * p_:64 * p_ + 64, :],
                        in_=bass.AP(tensor=base.tensor, offset=base.offset + h * N,
                                    ap=[[0, 64], [1, N]]),
                    )
                for p_ in range(2):
                    h = 2 * j + p_
                    pb = 64 * p_
                    nc.vector.tensor_tensor(
                        out=zt[j][pb:pb + 64, :],
                        in0=zun[pb:pb + 64, h * N:(h + 1) * N],
                        in1=rbc[pb:pb + 64, :], op=Mult,
                    )

            def out_proj_tail(s, ets, zt):
                """Out-proj for two e-tiles with the dd=3 (last head pair)
                contraction deferred, so these matmuls start before the last
                norm_pair's TT-mults have produced zt block 3."""
                pss = {}
                for et in ets:
                    ps = ps_s.tile([P, N], f32, tag="s", name=f"ps_ot_{s}_{et}")
                    pss[et] = ps
                    for dd in range(NDT - 1):
                        for qh in range(NQH):
                            nc.tensor.matmul(
                                ps[:, qh * 512:(qh + 1) * 512],
                                lhsT=w_sb["wo"][:, dd * 512 + et * P: dd * 512 + (et + 1) * P],
                                rhs=zt[dd][:, qh * 512: qh * 512 + 512],
                                start=(dd == 0), stop=False,
                            )
                for et in ets:
                    ps = pss[et]
                    dd = NDT - 1
                    for qh in range(NQH):
                        nc.tensor.matmul(
                            ps[:, qh * 512:(qh + 1) * 512],
                            lhsT=w_sb["wo"][:, dd * 512 + et * P: dd * 512 + (et + 1) * P],
                            rhs=zt[dd][:, qh * 512: qh * 512 + 512],
                            start=False, stop=True,
                        )
                    o_sb = opool.tile([P, N], f32, tag="o", name=f"o_{s}_{et}")
                    for qh in range(NQH):
                        nc.vector.tensor_scalar_add(
                            o_sb[:, qh * 512:(qh + 1) * 512],
                            ps[:, qh * 512:(qh + 1) * 512], b_sb["bo"][:, et:et + 1])
                    nc.sync.dma_start(out=OT[s, et * P:(et + 1) * P, :], in_=o_sb)

            def out_proj_chunk(s, et, zt):
                ps = ps_s.tile([P, N], f32, tag="s", name=f"ps_o_{s}_{et}")
                for dd in range(NDT):
                    for qh in range(NQH):
                        nc.tensor.matmul(
                            ps[:, qh * 512:(qh + 1) * 512],
                            lhsT=w_sb["wo"][:, dd * 512 + et * P: dd * 512 + (et + 1) * P],
                            rhs=zt[:, dd * N + qh * 512: dd * N + qh * 512 + 512],
                            start=(dd == 0), stop=(dd == NDT - 1),
                        )
                o_sb = opool.tile([P, N], f32, tag="o", name=f"o_{s}_{et}")
                for qh in range(NQH):
                    nc.vector.tensor_scalar_add(
                        o_sb[:, qh * 512:(qh + 1) * 512],
                        ps[:, qh * 512:(qh + 1) * 512], b_sb["bo"][:, et:et + 1])
                nc.sync.dma_start(out=OT[s, et * P:(et + 1) * P, :], in_=o_sb)

            def alloc_attn(s):
                zun = zpool.tile([P, H * N], bf16, tag="zun", name=f"zun_{s}")
                zt = [zpool.tile([P, N], bf16, tag=f"zt{j}", name=f"zt_{s}_{j}")
                      for j in range(NDT)]
                dall8 = rpool.tile([P, 64], f32, tag="dall", name=f"dall_{s}")
                rall8 = rpool.tile([P, 64], f32, tag="rall", name=f"rall_{s}")
                rdram = drpool.tile([P, 64], bf16, tag="rdram", name=f"rdram_{s}")
                return zun, zt, dall8, rall8, rdram

            # ---- schedule ----
            from itertools import chain

            emit_w("wq", WQT)
            xt0 = load_x(0)
            emit_w("wk", WKT)
            emit_b("bq", BQ)
            emit_b("bk", BK)
            emit_w("wv", WVT)
            emit_w("wo", WOT)
            emit_b("bo", BO)
            xt1 = load_x(1)

            # warm the PE HAM clock-gate while the input DMAs are in flight
            warm = wpool.tile([P, 512], bf16, tag="warm", name="warm_t")
            nc.vector.memset(warm, 0.0)
            warm_ps = ps_c.tile([P, N], f32, tag="c", name="warm_ps")
            for _ in range(20):
                nc.tensor.matmul(warm_ps[:, 0:512], lhsT=warm[:, 0:P], rhs=warm,
                                 start=True, stop=True)

            q0 = [qkpool.tile([P, N], bf16, tag=f"qt{j}", name=f"qt_0_{j}") for j in range(NDT)]
            k0 = [qkpool.tile([P, N], bf16, tag=f"kt{j}", name=f"kt_0_{j}") for j in range(NDT)]
            q1 = [qkpool.tile([P, N], bf16, tag=f"qt{j}", name=f"qt_1_{j}") for j in range(NDT)]
            k1 = [qkpool.tile([P, N], bf16, tag=f"kt{j}", name=f"kt_1_{j}") for j in range(NDT)]

            # startup (ACT idle): first QK chunk + BOTH slices' V projections
            proj_qk_chunk(0, 0, xt0, q0, k0)
            v0 = proj_v(0, xt0)
            v1 = proj_v(1, xt1)
            a0 = alloc_attn(0)
            a1 = alloc_attn(1)

            # filler chain: exactly 8 pairs x 16 kv-steps = 128 matmuls
            F = chain(
                gen_qk_chunk(0, 1, xt0, q0, k0),
                gen_qk_chunk(0, 2, xt0, q0, k0),
                gen_qk_chunk(0, 3, xt0, q0, k0),
                gen_qk_chunk(1, 0, xt1, q1, k1),
                gen_qk_chunk(1, 1, xt1, q1, k1),
                gen_qk_chunk(1, 2, xt1, q1, k1),
                gen_qk_chunk(1, 3, xt1, q1, k1),
                gen_op_chunk(0, 0, a0[1]),
                gen_op_chunk(0, 1, a0[1]),
            )

            for j in range(NDT):
                attention_pair(0, j, q0, k0, v0, a0[0], a0[2], filler=F)
                norm_pair(0, j, a0[0], a0[2], a0[3], a0[4], a0[1])
            for j in range(NDT - 1):
                attention_pair(1, j, q1, k1, v1, a1[0], a1[2], filler=F)
                norm_pair(1, j, a1[0], a1[2], a1[3], a1[4], a1[1])
            attention_pair(1, 3, q1, k1, v1, a1[0], a1[2], filler=F)
            norm_pair(1, 3, a1[0], a1[2], a1[3], a1[4], a1[1])
            for _ in F:  # drain any leftover fillers
                pass
            for _ in gen_op_chunk(0, 2, a0[1]):
                pass
            for _ in gen_op_chunk(0, 3, a0[1]):
                pass

            # tail out-proj: all dd<3 accumulations first (3 psum slots:
            # 2x ps_s + 1x ps_c), then the zt[3]-gated dd=3 closers, so the
            # PE's in-order stream never parks behind the last TT-norm.
            pss = {}
            for et, pool in ((0, ps_s), (1, ps_s), (2, ps_c)):
                ps = pool.tile([P, N], f32, tag=("s" if pool is ps_s else "c"),
                               name=f"ps_tl_{et}")
                pss[et] = ps
                for dd in range(NDT - 1):
                    for qh in range(NQH):
                        nc.tensor.matmul(
                            ps[:, qh * 512:(qh + 1) * 512],
                            lhsT=w_sb["wo"][:, dd * 512 + et * P: dd * 512 + (et + 1) * P],
                            rhs=a1[1][dd][:, qh * 512: qh * 512 + 512],
                            start=(dd == 0), stop=False,
                        )

            def _close(et):
                ps = pss[et]
                dd = NDT - 1
                for qh in range(NQH):
                    nc.tensor.matmul(
                        ps[:, qh * 512:(qh + 1) * 512],
                        lhsT=w_sb["wo"][:, dd * 512 + et * P: dd * 512 + (et + 1) * P],
                        rhs=a1[1][dd][:, qh * 512: qh * 512 + 512],
                        start=False, stop=True,
                    )
                o_sb = opool.tile([P, N], f32, tag="o", name=f"o_tl_{et}")
                for qh in range(NQH):
                    nc.vector.tensor_scalar_add(
                        o_sb[:, qh * 512:(qh + 1) * 512],
                        ps[:, qh * 512:(qh + 1) * 512], b_sb["bo"][:, et:et + 1])
                nc.sync.dma_start(out=OT[1, et * P:(et + 1) * P, :], in_=o_sb)

            _close(0)
            # et=3 accumulator reuses et=0's freed ps_s slot
            ps3 = ps_s.tile([P, N], f32, tag="s", name="ps_tl_3")
            pss[3] = ps3
            for dd in range(NDT - 1):
                for qh in range(NQH):
                    nc.tensor.matmul(
                        ps3[:, qh * 512:(qh + 1) * 512],
                        lhsT=w_sb["wo"][:, dd * 512 + 3 * P: dd * 512 + 4 * P],
                        rhs=a1[1][dd][:, qh * 512: qh * 512 + 512],
                        start=(dd == 0), stop=False,
                    )
            _close(1)
            _close(2)
            _close(3)

    nc.compile()
    return nc


def _get_nc():
    if "nc" not in _CACHE:
        _CACHE["nc"] = _build_nc()
    return _CACHE["nc"]


def kernel(X, Wq, bq, Wk, bk, Wv, bv, Wo, bo):
    from concourse.bass_utils import run_bass_kernel_spmd

    nc = _get_nc()
    bf16 = ml_dtypes.bfloat16

    Xf = np.asarray(X, np.float32).reshape(B * T, N, D)
    XT_all = np.ascontiguousarray(Xf.transpose(0, 2, 1)).astype(bf16)  # [16, D, N]
    WQT = np.ascontiguousarray(np.asarray(Wq, np.float32).T * S_SCALE).astype(bf16)
    WKT = np.ascontiguousarray(np.asarray(Wk, np.float32).T).astype(bf16)
    WVT = np.ascontiguousarray(np.asarray(Wv, np.float32).T).astype(bf16)
    WOT = np.ascontiguousarray(np.asarray(Wo, np.float32).T).astype(bf16)
    bo_eff = (np.asarray(bo, np.float32)
              + np.asarray(Wo, np.float32) @ np.asarray(bv, np.float32))
    BQa = (np.asarray(bq, np.float32) * S_SCALE).reshape(NDT, P, 1)
    BKa = np.asarray(bk, np.float32).reshape(NDT, P, 1)
    BOa = bo_eff.reshape(NDT, P, 1)

    in_maps = []
    for c in range(NCORES):
        in_maps.append({
            "XT": np.ascontiguousarray(XT_all[c * NSLICE:(c + 1) * NSLICE]),
            "WQT": WQT, "WKT": WKT, "WVT": WVT, "WOT": WOT,
            "BQ": BQa, "BK": BKa, "BO": BOa,
        })

    trace = bool(int(os.environ.get("KERNEL_TRACE", "0")))
    kwargs = {}
    if trace:
        import tempfile
        kwargs = {"trace": True, "tmpdir": tempfile.mkdtemp(prefix="ker_trace_")}
    res = run_bass_kernel_spmd(nc, in_maps, core_ids=list(range(NCORES)), **kwargs)
    _CACHE["last_exec_ns"] = res.exec_time_ns

    out = np.empty((B * T, N, D), np.float32)
    for c in range(NCORES):
        ot = np.asarray(res.results[c]["OT"], np.float32)  # [NSLICE, D, N]
        out[c * NSLICE:(c + 1) * NSLICE] = ot.transpose(0, 2, 1)
    return out.reshape(B, T, N, D)
